# revision 1
# baseline (speedup 1.0000x reference)
"""Bass/Tile TRN2 kernel for nn_Disen_GAT_For_Multi_Aspect.

Contract: kernel(**inputs) takes FULL fp32 numpy inputs (keys as in
reference.setup_inputs()) and returns the FULL [B, A, H] fp32 output.

Strategy
--------
Data-parallel over batch B across the 8 cores (1 batch row / core, A=4
aspects per core).  Heavy algebraic restructuring of the reference:

  q   = Wq^T asp + bq                          (per aspect, [H])
  u   = transA q ; v = transB q ; y = W1b v ; a3 = W1a^T q
  QT  = einsum(q, T1)   -> w = QT^T v          (one batched T1 pass)
  G   = Wk @ {q,w,y,u}                         ([D, k] per aspect)

  logits collapse to matvec rows against the raw (transposed) inputs:
    ST = (Wk q).T_n,  SX = (Wk{q,w,y}).X_n,  SD = (Wk u).Dp_n
  combined with mask-power rows M^k/SCALE and scalar bias terms.

  Only V_W = Wv^T X^T and V_T = Wv^T T^T are materialized [H,N];
  att_z[h] = sum_n r[n] V_W[h,n] V_T[h,n],  r = att_row * M^3.

Big streams (X^T, T^T, Dp^T, T1) ship as bf16; fp32 PSUM accumulate.
Phasing: V/ST/SD streams for aspects 0-1 run before the T1 pass (they
don't depend on it) to keep the PE warm; row algebra is aspect-stacked
into [4, 512] tiles so DVE ops use 4 lanes and 1/4 the instructions.
Constants ship as two packed tensors (one f32, one bf16) = 2 DMAs, and
DMA issue is spread across the sync/vector/scalar/gpsimd sequencers.
"""

import contextlib
import ctypes
import sys
import types

import numpy as np
import ml_dtypes

import concourse.bacc as bacc
import concourse.mybir as mybir
import concourse.tile as tile
from concourse.bass_utils import run_bass_kernel_spmd

B, A, N, D, H = 8, 4, 512, 1024, 128
SCALE = float(np.sqrt(H))
NCORES = 8
DC = D // H  # 8 contraction chunks of 128

F32 = mybir.dt.float32
BF16 = mybir.dt.bfloat16
BF = ml_dtypes.bfloat16
AF = mybir.ActivationFunctionType
OP = mybir.AluOpType

# cpack (f32) column layout
C_WKT = 0              # [128, 1024]  Wk^T
C_WQ = 1024            # [128, 8, 128] Wq chunk-packed
C_TAT = 2048           # transA^T
C_TBT = 2176           # transB^T
C_W1A = 2304           # W1a (raw)
C_W1BT = 2432          # W1b^T
C_BIAS = 2560          # [bq|bk|bv|b1|tb]
C_ASP = 2565           # [128, 8, 4] aspect^T chunk-packed
C_MASK = 2597          # rows 0-3: fmask replicated [4, 512]
C_COMBW = 3109         # rows 0-3: comb_w replicated [4, 3]
C_W = 3112
# bpack (bf16) column layout
B_WV = 0               # [128, 8, 128] Wv chunk-packed
B_E4 = 1024            # rows 0-3: E4[k, a*128+p] = (k == a)
B_W = 1536

LAST_RESULTS = None  # test harness peeks at this


def _build():
    nc = bacc.Bacc("TRN2", target_bir_lowering=False, debug=False,
                   num_devices=NCORES)

    xtd = nc.dram_tensor("xtd", [A, 128, DC, 3, N], BF16,
                         kind="ExternalInput")
    t1f = nc.dram_tensor("t1f", [H, H * H], mybir.dt.float8e4, kind="ExternalInput")
    cpackd = nc.dram_tensor("cpack", [128, C_W], F32, kind="ExternalInput")
    cpackh = nc.dram_tensor("cpackh", [128, 1057], F32, kind="ExternalInput")
    bpackd = nc.dram_tensor("bpack", [128, B_W], BF16, kind="ExternalInput")
    out = nc.dram_tensor("out", [H, A], F32, kind="ExternalOutput")
    qtf_dram = nc.dram_tensor("qtf_dram", [A, H * H], BF16)

    with tile.TileContext(nc) as tc:
        with (
            tc.tile_pool(name="const", bufs=1) as cp,
            tc.tile_pool(name="stream", bufs=2) as sp,
            tc.tile_pool(name="vwt", bufs=2) as vp,
            tc.tile_pool(name="rows", bufs=2) as rp,
            tc.tile_pool(name="big", bufs=2) as bp,
            tc.tile_pool(name="t1p", bufs=4) as t1p,
            tc.tile_pool(name="vps", bufs=4, space="PSUM") as vps,
            tc.tile_pool(name="sxps", bufs=3, space="PSUM") as rps,
            tc.tile_pool(name="smallps", bufs=1, space="PSUM") as sps,
        ):
            # ---- packed constants (2 DMAs) + T1 ----------------------
            cph = cp.tile([128, 1057], F32, tag="cph")
            nc.sync.dma_start(out=cph, in_=cpackh.ap())
            cpk = cp.tile([128, C_W], F32, tag="cpk")
            nc.sync.dma_start(out=cpk, in_=cpackd.ap())
            bpk = cp.tile([128, B_W], BF16, tag="bpk")
            nc.sync.dma_start(out=bpk, in_=bpackd.ap())

            wkt_sb = cpk[:, C_WKT:C_WKT + D]
            wq_v = cph[:, 0:DC * H].rearrange("p (c h) -> p c h", c=DC)
            tat_sb = cpk[:, C_TAT:C_TAT + H]
            tbt_sb = cpk[:, C_TBT:C_TBT + H]
            w1a_sb = cpk[:, C_W1A:C_W1A + H]
            w1bt_sb = cpk[:, C_W1BT:C_W1BT + H]
            bq_c = cph[:, 1056:1057]
            bk_c = cpk[:, C_BIAS + 1:C_BIAS + 2]
            bv_c = cpk[:, C_BIAS + 2:C_BIAS + 3]
            b1_c = cpk[:, C_BIAS + 3:C_BIAS + 4]
            tb_c = cpk[:, C_BIAS + 4:C_BIAS + 5]
            asp_v = cph[:, DC * H:DC * H + DC * A].rearrange(
                "p (c a) -> p c a", c=DC)
            mrep = cpk[0:4, C_MASK:C_MASK + N]       # [4, N]
            combw4 = cpk[0:4, C_COMBW:C_COMBW + 3]   # [4, 3]
            wv_v = bpk[:, B_WV:B_WV + DC * H].rearrange(
                "p (c h) -> p c h", c=DC)

            ones_col = cp.tile([128, 1], F32, tag="ones_col")
            nc.vector.memset(ones_col, 1.0)

            # ---- mask-power rows, all [4, N] -------------------------
            inv_s = 1.0 / SCALE
            neg_r = cp.tile([4, N], F32, tag="neg_r")
            nc.vector.tensor_scalar(neg_r, mrep, 1e30, 1e30,
                                    op0=OP.mult, op1=OP.subtract)

            # ---- small chain (T1-independent part) -------------------
            ps_q = sps.tile([H, A], F32, tag="sps")
            for c in range(DC):
                nc.tensor.matmul(ps_q, lhsT=wq_v[:, c, :],
                                 rhs=asp_v[:, c, :],
                                 start=(c == 0), stop=(c == DC - 1))
            q4 = cp.tile([H, A], F32, tag="q4")
            nc.scalar.activation(q4, ps_q, AF.Identity, bias=bq_c)
            q4bf = cp.tile([H, A], BF16, tag="q4bf")
            nc.vector.tensor_copy(q4bf, q4)
            q4f8 = cp.tile([H, A], mybir.dt.float8e4, tag="q4f8")
            nc.vector.tensor_copy(q4f8, q4)

            # qwyu: [q|w|y|u] x 4 aspects (w filled after the T1 pass)
            qwyu = cp.tile([H, 16], F32, tag="qwyu")
            nc.vector.tensor_copy(qwyu[:, 0:4], q4)

            ps_s = sps.tile([H, A], F32, tag="sps")
            nc.tensor.matmul(ps_s, lhsT=tbt_sb, rhs=q4, start=True, stop=True)
            v4 = cp.tile([H, A], F32, tag="v4")
            nc.vector.tensor_copy(v4, ps_s)
            v4bf = cp.tile([H, A], BF16, tag="v4bf")
            nc.vector.tensor_copy(v4bf, ps_s)

            ps_s = sps.tile([H, A], F32, tag="sps")
            nc.tensor.matmul(ps_s, lhsT=tat_sb, rhs=q4, start=True, stop=True)
            nc.vector.tensor_copy(qwyu[:, 12:16], ps_s)  # u

            ps_s = sps.tile([H, A], F32, tag="sps")
            nc.tensor.matmul(ps_s, lhsT=w1bt_sb, rhs=v4, start=True,
                             stop=True)
            nc.vector.tensor_copy(qwyu[:, 8:12], ps_s)  # y

            ps_s = sps.tile([H, A], F32, tag="sps")
            nc.tensor.matmul(ps_s, lhsT=w1a_sb, rhs=q4, start=True, stop=True)
            a3q = cp.tile([H, A], F32, tag="a3q")
            nc.vector.tensor_copy(a3q, ps_s)

            # gE = Wk @ [q|u]  (T1-independent; feeds ST and SD)
            qu8 = cp.tile([H, 8], F32, tag="qu8")
            nc.vector.tensor_copy(qu8[:, 0:4], q4)
            nc.vector.tensor_copy(qu8[:, 4:8], qwyu[:, 12:16])
            gE = cp.tile([128, DC, 8], BF16, tag="gE")
            for c in range(DC):
                ps_g = sps.tile([128, 8], F32, tag="sps")
                nc.tensor.matmul(ps_g, lhsT=wkt_sb[:, c * H:(c + 1) * H],
                                 rhs=qu8, start=True, stop=True)
                nc.scalar.copy(gE[:, c, :], ps_g)
            # gE[:, c, 0:4] = Wk q (ST), gE[:, c, 4:8] = Wk u (SD)
            gEv = gE[:, :, :].rearrange("p c (v a) -> p c v a", v=2)

            # ---- per-aspect stream machinery -------------------------
            va_tiles = {}
            rows5 = cp.tile([4, 5 * N], F32, tag="rows5")
            # layout per aspect row: [st | sd | sx0 | sx1 | sx2]

            def stream_vstsd(a, xa):
                # xa: [128, DC, 3, N] interleaved [X|T|Dp]
                ps_vw = vps.tile([H, N], F32, tag="vps")
                ps_vt = vps.tile([H, N], F32, tag="vps")
                ps_td = rps.tile([2, N], F32, tag="rps")
                ps_sd = rps.tile([2, N], F32, tag="rps")
                for c in range(DC):
                    nc.tensor.matmul(ps_vw, lhsT=wv_v[:, c, :],
                                     rhs=xa[:, c, 0, :], start=(c == 0),
                                     stop=(c == DC - 1))
                    nc.tensor.matmul(ps_vt, lhsT=wv_v[:, c, :],
                                     rhs=xa[:, c, 1, :], start=(c == 0),
                                     stop=(c == DC - 1))
                    nc.tensor.matmul(ps_td, lhsT=gEv[:, c, :, a],
                                     rhs=xa[:, c, 1, :], start=(c == 0),
                                     stop=(c == DC - 1))
                    nc.tensor.matmul(ps_sd, lhsT=gEv[:, c, :, a],
                                     rhs=xa[:, c, 2, :], start=(c == 0),
                                     stop=(c == DC - 1))
                vv_a = vp.tile([H, 2 * N], F32, tag="vwt")
                nc.scalar.activation(vv_a[:, 0:N], ps_vw, AF.Identity,
                                     bias=bv_c)
                nc.scalar.activation(vv_a[:, N:2 * N], ps_vt, AF.Identity,
                                     bias=bv_c)
                tdst = rp.tile([2, 2 * N], F32, tag="tdst", bufs=1)
                nc.vector.tensor_copy(tdst[:, 0:N], ps_td)
                nc.vector.tensor_copy(tdst[:, N:2 * N], ps_sd)
                nc.scalar.dma_start(out=rows5[a:a + 1, 0:N],
                                    in_=tdst[0:1, 0:N])
                nc.scalar.dma_start(out=rows5[a:a + 1, N:2 * N],
                                    in_=tdst[1:2, N:2 * N])
                va_tiles[a] = vv_a

            def stream_sx(a, xa, g4v):
                ps_sx = rps.tile([3, N], F32, tag="rps")
                for c in range(DC):
                    nc.tensor.matmul(ps_sx, lhsT=g4v[:, c, :, a],
                                     rhs=xa[:, c, 0, :], start=(c == 0),
                                     stop=(c == DC - 1))
                sxs = rp.tile([3, N], F32, tag="sxs")
                nc.vector.tensor_copy(sxs, ps_sx)
                for r in range(3):
                    nc.sync.dma_start(
                        out=rows5[a:a + 1, (2 + r) * N:(3 + r) * N],
                        in_=sxs[r:r + 1, :])

            # ---- T1 pass: QT = q . T1, then w = QT^T v ---------------
            qstg = None
            t1_sb = None
            for mcol in range(H * H // 512):
                if mcol % 8 == 0:
                    t1_sb = t1p.tile([128, 8 * 512], mybir.dt.float8e4,
                                     tag="t1")
                    pb = mcol * 512
                    nc.scalar.dma_start(out=t1_sb,
                                          in_=t1f.ap()[:, pb:pb + 8 * 512])
                ps_r = rps.tile([A, 512], F32, tag="rps")
                toff = (mcol % 8) * 512
                nc.tensor.matmul(ps_r, lhsT=q4f8,
                                 rhs=t1_sb[:, toff:toff + 512],
                                 start=True, stop=True)
                if mcol % 4 == 0:
                    qstg = rp.tile([A, 4 * 512], BF16, tag="qstg")
                off = (mcol % 4) * 512
                nc.vector.tensor_copy(qstg[:, off:off + 512], ps_r)
                if mcol % 4 == 3:
                    base = (mcol - 3) * 512
                    nc.scalar.dma_start(
                        out=qtf_dram.ap()[:, base:base + 4 * 512], in_=qstg)

            for a in range(A):
                qta = cp.tile([H, H], BF16, tag=f"qta{a}")
                nc.scalar.dma_start(out=qta, in_=qtf_dram.ap()[a].rearrange(
                    "(j k) -> j k", j=H))
                ps_w = sps.tile([H, 1], F32, tag="sps")
                nc.tensor.matmul(ps_w, lhsT=qta, rhs=v4bf[:, a:a + 1],
                                 start=True, stop=True)
                nc.vector.tensor_copy(qwyu[:, 4 + a:5 + a], ps_w)  # w

            # g4 = Wk @ [q|w|y] for SX
            g4 = cp.tile([128, DC, 12], BF16, tag="g4")
            for c in range(DC):
                ps_g = sps.tile([128, 12], F32, tag="sps")
                nc.tensor.matmul(ps_g, lhsT=wkt_sb[:, c * H:(c + 1) * H],
                                 rhs=qwyu[:, 0:12], start=True, stop=True)
                nc.scalar.copy(g4[:, c, :], ps_g)
            g4v = g4[:, :, :].rearrange("p c (v a) -> p c v a", v=3)

            # ---- scalar bias terms -> cmat [4, 7] --------------------
            tmp28 = cp.tile([H, 28], F32, tag="tmp28")
            nc.vector.tensor_scalar_mul(tmp28[:, 0:4], q4, bk_c)
            nc.vector.tensor_scalar_mul(tmp28[:, 4:8], qwyu[:, 12:16], bk_c)
            nc.vector.tensor_scalar_mul(tmp28[:, 8:12], qwyu[:, 4:8], bk_c)
            nc.vector.tensor_scalar_mul(tmp28[:, 12:16], qwyu[:, 8:12], bk_c)
            nc.vector.tensor_mul(tmp28[:, 16:20], a3q, v4)
            nc.vector.tensor_scalar_mul(tmp28[:, 20:24], v4, b1_c)
            nc.vector.tensor_scalar_mul(tmp28[:, 24:28], q4, tb_c)
            cmat = cp.tile([4, 7], F32, tag="cmat")
            for g in range(7):
                ps_c = sps.tile([4, 1], F32, tag="sps")
                nc.tensor.matmul(ps_c, lhsT=tmp28[:, 4 * g:4 * g + 4],
                                 rhs=ones_col, start=True, stop=True)
                nc.vector.tensor_copy(cmat[:, g:g + 1], ps_c)
            # groups: 0 cbk, 1 cu, 2 cw, 3 cy, 4 c3, 5 c5, 6 c6

            # ---- unified streams: V/ST/SD/SX + early V-product -------
            pp_tiles = {}

            def stream_all(a):
                xa = sp.tile([128, DC, 3, N], BF16, tag="xs")
                nc.sync.dma_start(out=xa, in_=xtd.ap()[a])
                stream_vstsd(a, xa)
                stream_sx(a, xa, g4v)
                vv_a = va_tiles[a]
                pprod = bp.tile([H, N], F32, tag="pp", bufs=4)
                nc.vector.tensor_mul(pprod, vv_a[:, 0:N], vv_a[:, N:2 * N])
                pp_tiles[a] = pprod

            attz_ref = [None]

            def finalpass():
                # ---- row algebra [4, N] (binary-mask collapsed) + finals -
                attz = cp.tile([H, A], F32, tag="attz")
                attz_ref[0] = attz
                zb4 = cp.tile([4, 1], F32, tag="zb4")
                nc.vector.memset(zb4, 0.0)
                st4 = rows5[:, 0:N]
                sd4 = rows5[:, N:2 * N]
                sxq = rows5[:, 2 * N:3 * N]
                sxw = rows5[:, 3 * N:4 * N]
                sxy = rows5[:, 4 * N:5 * N]
                cbk4 = cmat[:, 0:1]

                e_tw = rp.tile([4, N], F32, tag="e_tw")
                nc.vector.scalar_tensor_tensor(e_tw, st4, cbk4, mrep,
                                               op0=OP.add, op1=OP.mult)
                nc.vector.tensor_add(e_tw, e_tw, neg_r)
                z_tw = rp.tile([4, 1], F32, tag="z_tw")
                nc.scalar.activation(e_tw, e_tw, AF.Exp, bias=zb4,
                                     scale=inv_s, accum_out=z_tw)

                e_wi = rp.tile([4, N], F32, tag="e_wi")
                nc.vector.scalar_tensor_tensor(e_wi, sxq, cbk4, mrep,
                                               op0=OP.add, op1=OP.mult)
                nc.vector.tensor_add(e_wi, e_wi, neg_r)
                z_wi = rp.tile([4, 1], F32, tag="z_wi")
                nc.scalar.activation(e_wi, e_wi, AF.Exp, bias=zb4,
                                     scale=inv_s, accum_out=z_wi)

                # fmask is binary -> all DW mask powers collapse to m
                cdw = rp.tile([4, 1], F32, tag="cdw")
                nc.vector.tensor_add(cdw, cmat[:, 1:2], cmat[:, 5:6])
                nc.vector.tensor_add(cdw, cdw, cmat[:, 2:3])
                nc.vector.tensor_add(cdw, cdw, cmat[:, 4:5])
                nc.vector.tensor_add(cdw, cdw, cmat[:, 3:4])
                nc.vector.tensor_add(cdw, cdw, cmat[:, 6:7])
                e_dw = rp.tile([4, N], F32, tag="e_dw")
                nc.vector.tensor_add(e_dw, sd4, sxw)
                nc.vector.tensor_add(e_dw, e_dw, sxy)
                nc.vector.scalar_tensor_tensor(e_dw, e_dw, cdw, mrep,
                                               op0=OP.add, op1=OP.mult)
                nc.vector.tensor_add(e_dw, e_dw, neg_r)
                z_dw = rp.tile([4, 1], F32, tag="z_dw")
                nc.scalar.activation(e_dw, e_dw, AF.Exp, bias=zb4,
                                     scale=inv_s, accum_out=z_dw)

                zmat = rp.tile([4, 3], F32, tag="zmat")
                nc.vector.tensor_copy(zmat[:, 0:1], z_tw)
                nc.vector.tensor_copy(zmat[:, 1:2], z_wi)
                nc.vector.tensor_copy(zmat[:, 2:3], z_dw)
                rz = rp.tile([4, 3], F32, tag="rz")
                nc.vector.reciprocal(rz, zmat)
                alpha = rp.tile([4, 3], F32, tag="alpha")
                nc.vector.tensor_mul(alpha, rz, combw4)

                att = rp.tile([4, N], F32, tag="att")
                nc.vector.tensor_scalar_mul(att, e_tw, alpha[:, 0:1])
                nc.vector.scalar_tensor_tensor(att, e_wi, alpha[:, 1:2], att,
                                               op0=OP.mult, op1=OP.add)
                nc.vector.scalar_tensor_tensor(att, e_dw, alpha[:, 2:3], att,
                                               op0=OP.mult, op1=OP.add)
                rbf4 = rp.tile([4, N], BF16, tag="rbf4")
                nc.vector.tensor_mul(rbf4, att, mrep)

                for a in range(A):
                    ps_rb = sps.tile([H, N], F32, tag="sps")
                    nc.tensor.matmul(
                        ps_rb, lhsT=bpk[0:4, B_E4 + a * H:B_E4 + (a + 1) * H],
                        rhs=rbf4, start=True, stop=True)
                    p2 = bp.tile([H, N], F32, tag="pp2")
                    nc.vector.tensor_tensor(p2, pp_tiles[a], ps_rb, op=OP.mult)
                    nc.vector.reduce_sum(attz[:, a:a + 1], p2,
                                         axis=mybir.AxisListType.X)

            stream_all(0)
            stream_all(1)
            stream_all(2)
            stream_all(3)
            finalpass()
            nc.sync.dma_start(out=out.ap(), in_=attz_ref[0])

    nc.compile()
    return nc


def _prep_inputs(inputs):
    f = {k: np.asarray(v, dtype=np.float32) for k, v in inputs.items()}
    cpack = np.zeros((128, C_W), np.float32)
    cpack[:, C_WKT:C_WKT + D] = f["Wk"].T
    cpack[:, C_WQ:C_WQ + DC * H] = np.transpose(
        f["Wq"].reshape(DC, 128, H), (1, 0, 2)).reshape(128, DC * H)
    cpack[:, C_TAT:C_TAT + H] = f["trans_W"][:H].T
    cpack[:, C_TBT:C_TBT + H] = f["trans_W"][H:].T
    cpack[:, C_W1A:C_W1A + H] = f["W1_W"][:H]
    cpack[:, C_W1BT:C_W1BT + H] = f["W1_W"][H:].T
    for i, k in enumerate(("bq", "bk", "bv", "W1_b", "trans_b")):
        cpack[:, C_BIAS + i] = f[k]
    cpack[0:4, C_COMBW:C_COMBW + 3] = np.tile(f["comb_w"], (4, 1))

    bpack = np.zeros((128, B_W), np.float32)
    bpack[:, B_WV:B_WV + DC * H] = np.transpose(
        f["Wv"].reshape(DC, 128, H), (1, 0, 2)).reshape(128, DC * H)
    for a in range(A):
        bpack[a, B_E4 + a * H:B_E4 + (a + 1) * H] = 1.0
    bpack = bpack.astype(BF)

    shared = {"t1f": f["T1"].reshape(H, H * H).astype(ml_dtypes.float8_e4m3fn), "bpack": bpack}
    in_maps = []
    for b in range(NCORES):
        cp_b = cpack.copy()
        cp_b[:, C_ASP:C_ASP + DC * A] = np.transpose(
            f["aspect_feature"][b].T.reshape(DC, 128, A),
            (1, 0, 2)).reshape(128, DC * A)
        cp_b[0:4, C_MASK:C_MASK + N] = np.tile(f["fmask"][b], (4, 1))
        m = dict(shared)
        m["cpack"] = cp_b
        cph_b = np.zeros((128, 1057), np.float32)
        cph_b[:, 0:DC * H] = cp_b[:, C_WQ:C_WQ + DC * H]
        cph_b[:, DC * H:DC * H + DC * A] = cp_b[:, C_ASP:C_ASP + DC * A]
        cph_b[:, 1056] = f["bq"]
        m["cpackh"] = cph_b
        xs = np.stack([f["feature"][b], f["all_type_feature"][b],
                       f["dep_feature"][b]], axis=2)  # [A, N, 3, D]
        # -> [A, 128(p), DC(c), 3, N]: element (a,p,c,s,n) = xs[a,n,s,c*128+p]
        m["xtd"] = np.ascontiguousarray(
            xs.transpose(0, 3, 2, 1).reshape(A, DC, 128, 3, N)
              .transpose(0, 2, 1, 3, 4)).astype(BF)
        in_maps.append(m)
    return in_maps


def _install_ntff_shim():
    """Provide antenv.axon_hooks (absent in this image) so trace=True can
    drive NTFF capture through libaxon_pjrt.so."""
    if "antenv.axon_hooks" in sys.modules:
        return
    import antenv

    mod = types.ModuleType("antenv.axon_hooks")
    mod._hook = None
    mod.set_axon_ntff_profile_hook = lambda h: setattr(mod, "_hook", h)
    mod.get_axon_ntff_profile_hook = lambda: mod._hook
    sys.modules["antenv.axon_hooks"] = mod
    antenv.axon_hooks = mod

    so_path = "/opt/axon/libaxon_pjrt.so"
    try:
        lib = ctypes.CDLL(so_path)
    except OSError:
        return
    if not hasattr(lib, "axon_start_nrt_profile"):
        return
    lib.axon_start_nrt_profile.argtypes = [ctypes.POINTER(ctypes.c_int64),
                                           ctypes.c_size_t]
    lib.axon_start_nrt_profile.restype = ctypes.c_int64
    lib.axon_stop_nrt_profile.argtypes = [ctypes.c_char_p]
    lib.axon_stop_nrt_profile.restype = ctypes.c_int64

    @contextlib.contextmanager
    def _hook(output_dir, device_ids):
        import jax

        jax.devices()
        if device_ids:
            ids = (ctypes.c_int64 * len(device_ids))(*device_ids)
            rc = lib.axon_start_nrt_profile(ids, len(device_ids))
        else:
            rc = lib.axon_start_nrt_profile(None, 0)
        if rc != 0:
            raise RuntimeError(f"axon_start_nrt_profile rc={rc}")
        try:
            yield
        finally:
            n = lib.axon_stop_nrt_profile(str(output_dir).encode())
            print(f"profile: {n} file(s) written to {output_dir}")

    mod.set_axon_ntff_profile_hook(_hook)


def kernel(feature, dep_feature, aspect_feature, all_type_feature, fmask,
           Wq, bq, Wk, bk, Wv, bv, trans_W, trans_b, T1, W1_W, W1_b, comb_w,
           _profile=False, _tmpdir=None):
    global LAST_RESULTS
    inputs = dict(feature=feature, dep_feature=dep_feature,
                  aspect_feature=aspect_feature,
                  all_type_feature=all_type_feature, fmask=fmask, Wq=Wq,
                  bq=bq, Wk=Wk, bk=bk, Wv=Wv, bv=bv, trans_W=trans_W,
                  trans_b=trans_b, T1=T1, W1_W=W1_W, W1_b=W1_b,
                  comb_w=comb_w)
    nc = _build()
    in_maps = _prep_inputs(inputs)
    if _profile:
        _install_ntff_shim()
    res = run_bass_kernel_spmd(nc, in_maps, list(range(NCORES)),
                               trace=_profile, tmpdir=_tmpdir)
    LAST_RESULTS = res
    full = np.stack([res.results[c]["out"].T for c in range(NCORES)])
    return full.astype(np.float32)



# revision 10
# speedup vs baseline: 1.1866x; 1.1866x over previous
"""Bass/Tile TRN2 kernel for nn_Disen_GAT_For_Multi_Aspect (v3).

Contract: kernel(**inputs) takes FULL fp32 numpy inputs (keys as in
reference.setup_inputs()) and returns the FULL [B, A, H] fp32 output.

Strategy
--------
Data-parallel over batch B across the 8 cores (1 batch row / core, A=4
aspects per core).  The reference collapses algebraically:

  q = Wq^T asp + bq;  u = tA q; v = tB q; y = W1b v; a3 = W1a^T q
  w[a,k] = sum_{i,j} q[a,i] v[a,j] T1[i,j,k]
  G = Wk @ [q|w|y|u]                    (per aspect, [D,4])
  logit rows vs raw streams:  st = (Wk q).T_n, sx* = (Wk{q,w,y}).X_n,
                              sd = (Wk u).Dp_n
  V_W = Wv^T X + bv, V_T = Wv^T T + bv  ([H,N] per aspect)
  att_z[h] = sum_n att_n V_W[h,n] V_T[h,n]

PE-centric v3 layout:
 * T1 pass computes w DIRECTLY: 128 accumulating matmuls with rank-1
   lhsT chunks qv_j = q (.) v_j (fp8), rhs = T1[:, j-block] (fp8).
 * Per aspect the 5 logit rows land in ONE PSUM bank via tile_position
   col placement (X-rows@0, T-rows@32, Dp-rows@64, neg-row@96), padded
   to M=32 so every PSUM partition is written.  Row matmuls run in
   fp8 DoubleRow mode (K=256 chunk pairs): device-side single-copy
   casts of the bf16 X|T stream and of G to fp8.  One full-width bf16
   copy of the row bank, then softmax as: combo-matmul [128x3] -> ACT
   Exp(bias,scale,accum z) -> reciprocal -> alpha-broadcast matmul =
   att replicated on 128 partitions -> multiply+reduce vs V_W*V_T.
 * Streams: X,T bf16 (V path); rows read fp8 casts; Dp fp8; T1 fp8.
 * q-chain in bf16; constants split hot (q-chain) / cold (stream
   weights) so the first matmul issues as early as possible.
"""

import contextlib
import ctypes
import sys
import types

import numpy as np
import ml_dtypes

import concourse.bacc as bacc
import concourse.mybir as mybir
import concourse.tile as tile
from concourse.bass_utils import run_bass_kernel_spmd

B, A, N, D, H = 8, 4, 512, 1024, 128
SCALE = float(np.sqrt(H))
NCORES = 8
DC = D // H  # 8 contraction chunks of 128

F32 = mybir.dt.float32
BF16 = mybir.dt.bfloat16
F8 = mybir.dt.float8e4
BF = ml_dtypes.bfloat16
E4 = ml_dtypes.float8_e4m3fn
AF = mybir.ActivationFunctionType
OP = mybir.AluOpType
DR = mybir.MatmulPerfMode.DoubleRow

# cpackh (bf16, hot: q-chain weights) column layout
CH_WQ = 0              # [128, 8, 128] Wq chunk-packed
CH_ASP = 1024          # [128, 8, 4] aspect^T chunk-packed
CH_TAT = 1056          # trans_W[:H].T
CH_TBT = 1184          # trans_W[H:].T
CH_W1A = 1312          # W1_W[:H] (raw)
CH_W1BT = 1440         # W1_W[H:].T
CH_W = 1568
# cpackf (f32, small) column layout
CF_BQROW = 0           # rows 0-3: bq as a row [4, 128]
CF_BIAS = 128          # cols: bq|bk|bv|W1_b|trans_b
CF_COMBW = 133         # rows 0-2: comb_w column
CF_MASK4 = 134         # rows 0-3: eye(4) columns
CF_M01 = 138           # [1/S, 1/S, 0] column (rows 0-2)
CF_M2 = 139            # [0, 0, 1/S] column
CF_W = 140
# cpackb (bf16, cold: stream weights) column layout
CB_WKT = 0             # [128, 1024]  Wk^T
CB_WV = 1024           # [128, 8, 128] Wv chunk-packed
CB_MROW = 2048         # rows 0-3: fmask replicated [4, 512]
CB_COMBO = 2560        # [128, 3] combo matrix
CB_E0 = 2563           # [4, 32] one-hot row-0 picker (neg MM lhsT)
CB_ID4 = 2595          # [4, 4] identity
CB_W = 2600

LAST_RESULTS = None  # test harness peeks at this


def _build(ncores=NCORES):
    nc = bacc.Bacc("TRN2", target_bir_lowering=False, debug=False,
                   num_devices=ncores)

    xt = nc.dram_tensor("xt", [A, 128, DC, 2, N], BF16, kind="ExternalInput")
    dp8 = nc.dram_tensor("dp8", [A, 128, DC // 2, 2, N], F8,
                         kind="ExternalInput")
    t1f = nc.dram_tensor("t1f", [H, H * H], F8, kind="ExternalInput")
    cpackh = nc.dram_tensor("cpackh", [128, CH_W], BF16, kind="ExternalInput")
    cpackf = nc.dram_tensor("cpackf", [128, CF_W], F32, kind="ExternalInput")
    cpackb = nc.dram_tensor("cpackb", [128, CB_W], BF16, kind="ExternalInput")
    out = nc.dram_tensor("out", [H, A], F32, kind="ExternalOutput")

    inv_s = 1.0 / SCALE

    with tile.TileContext(nc) as tc:
        with (
            tc.tile_pool(name="const", bufs=1) as cp,
            tc.tile_pool(name="t1s", bufs=2) as tp,
            tc.tile_pool(name="xts", bufs=2) as xp,
            tc.tile_pool(name="work", bufs=2) as wp,
            tc.tile_pool(name="vzone", bufs=4, space="PSUM") as vps,
            tc.tile_pool(name="rzone", bufs=2, space="PSUM") as rps,
            tc.tile_pool(name="szone", bufs=1, space="PSUM") as sps,
        ):
            # ---- input DMAs, spread across the three DMA sequencers ---
            cph = cp.tile([128, CH_W], BF16, tag="cph")
            nc.sync.dma_start(out=cph, in_=cpackh.ap())
            cpf = cp.tile([128, CF_W], F32, tag="cpf")
            nc.scalar.dma_start(out=cpf, in_=cpackf.ap())
            t1sb = []
            for i in range(4):
                t = tp.tile([128, 4096], F8, tag="t1", bufs=4)
                eng = nc.sync if i % 2 == 0 else nc.scalar
                eng.dma_start(out=t, in_=t1f.ap()[:, 4096 * i:4096 * (i + 1)])
                t1sb.append(t)
            cpb = cp.tile([128, CB_W], BF16, tag="cpb")
            nc.scalar.dma_start(out=cpb, in_=cpackb.ap())
            xa_t = {}
            dp_t = {}
            xeng = [nc.gpsimd, nc.sync, nc.scalar, nc.gpsimd]
            deng = [nc.gpsimd, nc.gpsimd, nc.sync, nc.scalar]
            for a in range(A):
                xa = xp.tile([128, DC, 2, N], BF16, tag="xt", bufs=3)
                xeng[a].dma_start(out=xa, in_=xt.ap()[a])
                da = xp.tile([128, DC // 2, 2, N], F8, tag="dp")
                deng[a].dma_start(out=da, in_=dp8.ap()[a])
                xa_t[a] = xa
                dp_t[a] = da

            # ---- constant views ---------------------------------------
            wq_v = cph[:, CH_WQ:CH_WQ + DC * H].rearrange(
                "p (c h) -> p c h", c=DC)
            asp_v = cph[:, CH_ASP:CH_ASP + DC * A].rearrange(
                "p (c a) -> p c a", c=DC)
            tat_sb = cph[:, CH_TAT:CH_TAT + H]
            tbt_sb = cph[:, CH_TBT:CH_TBT + H]
            w1a_sb = cph[:, CH_W1A:CH_W1A + H]
            w1bt_sb = cph[:, CH_W1BT:CH_W1BT + H]
            bqrow = cpf[0:4, CF_BQROW:CF_BQROW + H]
            bq_c = cpf[:, CF_BIAS + 0:CF_BIAS + 1]
            bk_c = cpf[:, CF_BIAS + 1:CF_BIAS + 2]
            bv_c = cpf[:, CF_BIAS + 2:CF_BIAS + 3]
            b1_c = cpf[:, CF_BIAS + 3:CF_BIAS + 4]
            tb_c = cpf[:, CF_BIAS + 4:CF_BIAS + 5]
            combw3 = cpf[0:3, CF_COMBW:CF_COMBW + 1]
            mask4 = cpf[0:4, CF_MASK4:CF_MASK4 + 4]
            m01_c = cpf[0:3, CF_M01:CF_M01 + 1]
            m2_c = cpf[0:3, CF_M2:CF_M2 + 1]
            wkt_sb = cpb[:, CB_WKT:CB_WKT + D]
            wv_v = cpb[:, CB_WV:CB_WV + DC * H].rearrange(
                "p (c h) -> p c h", c=DC)
            mrow4 = cpb[0:4, CB_MROW:CB_MROW + N]
            combo_m = cpb[:, CB_COMBO:CB_COMBO + 3]
            id4 = cpb[0:4, CB_ID4:CB_ID4 + 4]

            ones_col = cp.tile([128, 1], F32, tag="ones_col")
            nc.vector.memset(ones_col, 1.0)
            ones3r = cp.tile([3, 128], BF16, tag="ones3r")
            nc.vector.memset(ones3r, 1.0)

            # neg rows: [32, N] with row 0 = -1e30*(1-m), rest zero
            negfull = cp.tile([32, N], BF16, tag="negfull")
            nc.vector.memset(negfull, 0.0)
            nc.vector.tensor_scalar(negfull[0:1, :], mrow4[0:1, :], 1e30,
                                    1e30, op0=OP.mult, op1=OP.subtract)

            # ---- q chain (bf16 matmuls, fp32 psum) --------------------
            ps_q = sps.tile([H, A], F32, tag="s")
            for c in range(DC):
                nc.tensor.matmul(ps_q, lhsT=wq_v[:, c, :], rhs=asp_v[:, c, :],
                                 start=(c == 0), stop=(c == DC - 1))
            q4 = cp.tile([H, A], F32, tag="q4")
            nc.scalar.activation(q4, ps_q, AF.Identity, bias=bq_c)
            q4b = cp.tile([H, A], BF16, tag="q4b")
            nc.vector.tensor_copy(q4b, q4)

            ps_qT = sps.tile([A, H], F32, tag="s")
            for c in range(DC):
                nc.tensor.matmul(ps_qT, lhsT=asp_v[:, c, :], rhs=wq_v[:, c, :],
                                 start=(c == 0), stop=(c == DC - 1))
            qTb = cp.tile([A, H], BF16, tag="qTb")
            nc.vector.tensor_tensor(qTb, ps_qT, bqrow, op=OP.add)

            ps_s = sps.tile([H, A], F32, tag="s")
            nc.tensor.matmul(ps_s, lhsT=tbt_sb, rhs=q4b, start=True, stop=True)
            v4 = cp.tile([H, A], F32, tag="v4")
            nc.vector.tensor_copy(v4, ps_s)
            v4b = cp.tile([H, A], BF16, tag="v4b")
            nc.vector.tensor_copy(v4b, ps_s)

            ps_vT = sps.tile([A, H], F32, tag="s")
            nc.tensor.matmul(ps_vT, lhsT=q4b, rhs=tbt_sb, start=True,
                             stop=True)
            vTb = cp.tile([A, H], BF16, tag="vTb")
            nc.vector.tensor_copy(vTb, ps_vT)

            # qwyu: aspect-major columns [q|w|y|u] per aspect, bf16
            qwyu = cp.tile([H, 16], BF16, tag="qwyu")
            qwv = qwyu.rearrange("p (a v) -> p a v", a=4)
            nc.vector.tensor_copy(qwv[:, :, 0], q4)

            ps_s = sps.tile([H, A], F32, tag="s")
            nc.tensor.matmul(ps_s, lhsT=tat_sb, rhs=q4b, start=True, stop=True)
            u4 = cp.tile([H, A], F32, tag="u4")
            nc.vector.tensor_copy(u4, ps_s)
            nc.vector.tensor_copy(qwv[:, :, 3], ps_s)

            ps_s = sps.tile([H, A], F32, tag="s")
            nc.tensor.matmul(ps_s, lhsT=w1bt_sb, rhs=v4b, start=True,
                             stop=True)
            y4 = cp.tile([H, A], F32, tag="y4")
            nc.vector.tensor_copy(y4, ps_s)
            nc.vector.tensor_copy(qwv[:, :, 2], ps_s)

            ps_s = sps.tile([H, A], F32, tag="s")
            nc.tensor.matmul(ps_s, lhsT=w1a_sb, rhs=q4b, start=True, stop=True)
            a3q = cp.tile([H, A], F32, tag="a3q")
            nc.vector.tensor_copy(a3q, ps_s)

            # ---- qv outer products (masked K=4), cast fp8 -------------
            ps_qv = sps.tile([128, 4 * H], F32, tag="s")
            for a in range(A):
                vTm = wp.tile([A, H], BF16, tag="vTm")
                nc.vector.tensor_scalar_mul(vTm, vTb, mask4[:, a:a + 1])
                nc.tensor.matmul(ps_qv[:, a * H:(a + 1) * H], lhsT=qTb,
                                 rhs=vTm, start=True, stop=True)
            qv8 = cp.tile([128, 4 * H], F8, tag="qv8")
            nc.vector.tensor_copy(qv8, ps_qv)
            qv8v = qv8.rearrange("p (a j) -> p j a", a=4)

            # ---- T1 pass: w[a,k] = sum_j qv_j . T1[:, j-block] --------
            ps_w = sps.tile([A, H], F32, tag="s")
            for j in range(H):
                nc.tensor.matmul(ps_w, lhsT=qv8v[:, j, :],
                                 rhs=t1sb[j // 32][:, (j % 32) * H:
                                                   (j % 32 + 1) * H],
                                 start=(j == 0), stop=(j == H - 1))
            wbf = cp.tile([A, H], BF16, tag="wbf")
            nc.vector.tensor_copy(wbf, ps_w)
            ps_tr = sps.tile([H, A], BF16, tag="s")
            nc.tensor.transpose(ps_tr, wbf, id4)
            nc.vector.tensor_copy(qwv[:, :, 1], ps_tr)

            # ---- scalar terms -> bias_all [3, A] ----------------------
            # groups: cbk | u.bk | w.bk | y.bk | a3.v | v.W1b | q.tb
            tmp28 = cp.tile([H, 28], F32, tag="tmp28")
            nc.vector.tensor_scalar_mul(tmp28[:, 0:4], q4, bk_c)
            nc.vector.tensor_scalar_mul(tmp28[:, 4:8], u4, bk_c)
            wcol = cp.tile([H, A], F32, tag="wcol")
            nc.vector.tensor_copy(wcol, ps_tr)
            nc.vector.tensor_scalar_mul(tmp28[:, 8:12], wcol, bk_c)
            nc.vector.tensor_scalar_mul(tmp28[:, 12:16], y4, bk_c)
            nc.vector.tensor_mul(tmp28[:, 16:20], a3q, v4)
            nc.vector.tensor_scalar_mul(tmp28[:, 20:24], v4, b1_c)
            nc.vector.tensor_scalar_mul(tmp28[:, 24:28], q4, tb_c)
            ps_c28 = sps.tile([1, 28], F32, tag="s")
            nc.tensor.matmul(ps_c28, lhsT=ones_col, rhs=tmp28,
                             start=True, stop=True)
            c28 = cp.tile([1, 28], F32, tag="c28")
            nc.vector.tensor_copy(c28, ps_c28)
            one13 = cp.tile([1, 3], F32, tag="one13")
            nc.vector.memset(one13, 1.0)
            ps_r3 = sps.tile([3, 28], F32, tag="s")
            nc.tensor.matmul(ps_r3, lhsT=one13, rhs=c28, start=True, stop=True)
            rep3 = cp.tile([3, 28], F32, tag="rep3")
            nc.vector.tensor_copy(rep3, ps_r3)
            cdw3 = cp.tile([3, A], F32, tag="cdw3")
            nc.vector.tensor_tensor(cdw3, rep3[:, 4:8], rep3[:, 8:12],
                                    op=OP.add)
            nc.vector.tensor_tensor(cdw3, cdw3, rep3[:, 12:16], op=OP.add)
            nc.vector.tensor_tensor(cdw3, cdw3, rep3[:, 16:20], op=OP.add)
            nc.vector.tensor_tensor(cdw3, cdw3, rep3[:, 20:24], op=OP.add)
            nc.vector.tensor_tensor(cdw3, cdw3, rep3[:, 24:28], op=OP.add)
            bias_all = cp.tile([3, A], F32, tag="bias_all")
            nc.vector.tensor_scalar_mul(bias_all, rep3[:, 0:4], m01_c)
            nc.vector.scalar_tensor_tensor(bias_all, cdw3, m2_c, bias_all,
                                           op0=OP.mult, op1=OP.add)

            # ---- G4 = Wk @ qwyu -> gall (zero-padded) + fp8 copy ------
            gall = cp.tile([128, DC, 48], BF16, tag="gall")
            nc.vector.memset(gall, 0.0)
            for c in range(DC):
                ps_g = sps.tile([128, 16], F32, tag="s")
                nc.tensor.matmul(ps_g, lhsT=wkt_sb[:, c * H:(c + 1) * H],
                                 rhs=qwyu, start=True, stop=True)
                nc.vector.tensor_copy(gall[:, c, 0:16], ps_g)
            g8 = cp.tile([128, DC, 48], F8, tag="g8")
            nc.vector.tensor_copy(g8, gall)
            g8p = g8.rearrange("p (c2 pair) f -> p c2 pair f", pair=2)

            # ---- per-aspect streams + finalization --------------------
            attz = cp.tile([H, A], F32, tag="attz")

            for a in range(A):
                xa, da = xa_t[a], dp_t[a]
                # fp8 cast of the whole X|T stream (layout-preserving)
                x8 = wp.tile([128, DC, 2, N], F8, tag="x8")
                nc.vector.tensor_copy(x8, xa)
                x8p = x8.rearrange("p (c2 pair) s n -> p c2 pair s n", pair=2)

                # row matmuls (fp8 DoubleRow, K=256), one bank per group
                grp = []
                for rhs_of in (lambda c2: x8p[:, c2, :, 0, :],
                               lambda c2: x8p[:, c2, :, 1, :],
                               lambda c2: da[:, c2, :, :]):
                    ps_r = rps.tile([32, N], F32, tag="rows", bufs=3)
                    for c2 in range(DC // 2):
                        nc.tensor.matmul(ps_r,
                                         lhsT=g8p[:, c2, :, 4 * a:4 * a + 32],
                                         rhs=rhs_of(c2), start=(c2 == 0),
                                         stop=(c2 == DC // 2 - 1),
                                         perf_mode=DR)
                    grp.append(ps_r)

                # assemble rows bank copy: X@0, T@32, Dp@64, neg@96
                rows_bf = wp.tile([128, N], BF16, tag="rows_bf")
                nc.vector.tensor_copy(rows_bf[0:32, :], grp[0])
                nc.scalar.copy(rows_bf[32:64, :], grp[1])
                nc.vector.tensor_copy(rows_bf[64:96, :], grp[2])
                nc.scalar.copy(rows_bf[96:128, :], negfull)
                ps_combo = sps.tile([3, N], F32, tag="s")
                nc.tensor.matmul(ps_combo, lhsT=combo_m, rhs=rows_bf,
                                 start=True, stop=True)
                e3 = wp.tile([3, N], BF16, tag="e3")
                z3 = wp.tile([3, 1], F32, tag="z3")
                nc.scalar.activation(e3, ps_combo, AF.Exp,
                                     bias=bias_all[:, a:a + 1], scale=inv_s,
                                     accum_out=z3)
                rz = wp.tile([3, 1], F32, tag="rz")
                nc.vector.reciprocal(rz, z3)
                alpha = wp.tile([3, 1], F32, tag="alpha")
                nc.vector.tensor_mul(alpha, rz, combw3)
                arep = wp.tile([3, H], BF16, tag="arep")
                nc.vector.tensor_scalar_mul(arep, ones3r, alpha)

                # V matmuls
                ps_vw = vps.tile([H, N], F32, tag="v")
                ps_vt = vps.tile([H, N], F32, tag="v")
                for c in range(DC):
                    nc.tensor.matmul(ps_vw, lhsT=wv_v[:, c, :],
                                     rhs=xa[:, c, 0, :], start=(c == 0),
                                     stop=(c == DC - 1))
                    nc.tensor.matmul(ps_vt, lhsT=wv_v[:, c, :],
                                     rhs=xa[:, c, 1, :], start=(c == 0),
                                     stop=(c == DC - 1))
                vv = wp.tile([H, 2 * N], F32, tag="vv")
                nc.scalar.activation(vv[:, 0:N], ps_vw, AF.Identity,
                                     bias=bv_c)
                nc.scalar.activation(vv[:, N:2 * N], ps_vt, AF.Identity,
                                     bias=bv_c)
                pprod = wp.tile([H, N], F32, tag="pprod")
                nc.vector.tensor_mul(pprod, vv[:, 0:N], vv[:, N:2 * N])

                ps_att = sps.tile([H, N], F32, tag="s")
                nc.tensor.matmul(ps_att, lhsT=arep, rhs=e3,
                                 start=True, stop=True)
                scr = wp.tile([H, N], F32, tag="scr")
                nc.vector.tensor_mul(scr, ps_att, pprod)
                nc.vector.tensor_reduce(attz[:, a:a + 1], scr,
                                        axis=mybir.AxisListType.X,
                                        op=OP.add)

            nc.sync.dma_start(out=out.ap(), in_=attz)

    nc.compile()
    return nc


def _prep_inputs(inputs):
    f = {k: np.asarray(v, dtype=np.float32) for k, v in inputs.items()}
    S = SCALE

    cpackh = np.zeros((128, CH_W), np.float32)
    cpackh[:, CH_WQ:CH_WQ + DC * H] = np.transpose(
        f["Wq"].reshape(DC, 128, H), (1, 0, 2)).reshape(128, DC * H)
    cpackh[:, CH_TAT:CH_TAT + H] = f["trans_W"][:H].T
    cpackh[:, CH_TBT:CH_TBT + H] = f["trans_W"][H:].T
    cpackh[:, CH_W1A:CH_W1A + H] = f["W1_W"][:H]
    cpackh[:, CH_W1BT:CH_W1BT + H] = f["W1_W"][H:].T

    cpackf = np.zeros((128, CF_W), np.float32)
    cpackf[0:4, CF_BQROW:CF_BQROW + H] = np.tile(f["bq"], (4, 1))
    for i, k in enumerate(("bq", "bk", "bv", "W1_b", "trans_b")):
        cpackf[:, CF_BIAS + i] = f[k]
    cpackf[0:3, CF_COMBW] = f["comb_w"]
    cpackf[0:4, CF_MASK4:CF_MASK4 + 4] = np.eye(4)
    cpackf[0:3, CF_M01] = [1.0 / S, 1.0 / S, 0.0]
    cpackf[0:3, CF_M2] = [0.0, 0.0, 1.0 / S]

    cpackb = np.zeros((128, CB_W), np.float32)
    cpackb[:, CB_WKT:CB_WKT + D] = f["Wk"].T
    cpackb[:, CB_WV:CB_WV + DC * H] = np.transpose(
        f["Wv"].reshape(DC, 128, H), (1, 0, 2)).reshape(128, DC * H)
    # combo matrix: ch0(TW): st@32, neg@96; ch1(Wi): sxq@0, neg@96;
    # ch2(DW): sxw@1, sxy@2, sd@67, neg@96
    cpackb[32, CB_COMBO + 0] = 1.0
    cpackb[96, CB_COMBO + 0] = 1.0
    cpackb[0, CB_COMBO + 1] = 1.0
    cpackb[96, CB_COMBO + 1] = 1.0
    cpackb[1, CB_COMBO + 2] = 1.0
    cpackb[2, CB_COMBO + 2] = 1.0
    cpackb[67, CB_COMBO + 2] = 1.0
    cpackb[96, CB_COMBO + 2] = 1.0
    cpackb[0, CB_E0] = 1.0
    cpackb[0:4, CB_ID4:CB_ID4 + 4] = np.eye(4)

    t1 = f["T1"].reshape(H, H * H)
    cpackh_bf = cpackh.astype(BF)
    t1_e4 = np.clip(t1, -240, 240).astype(E4)

    in_maps = []
    for b in range(NCORES):
        ch = cpackh_bf.copy()
        ch[:, CH_ASP:CH_ASP + DC * A] = np.transpose(
            f["aspect_feature"][b].T.reshape(DC, 128, A),
            (1, 0, 2)).reshape(128, DC * A).astype(BF)
        cb = cpackb.copy()
        cb[0:4, CB_MROW:CB_MROW + N] = np.tile(f["fmask"][b], (4, 1))
        m = {"t1f": t1_e4, "cpackh": ch, "cpackf": cpackf,
             "cpackb": cb.astype(BF)}
        xs = np.stack([f["feature"][b], f["all_type_feature"][b]], axis=2)
        # [A, N, 2, D] -> [A, 128(p), DC(c), 2, N]
        m["xt"] = np.ascontiguousarray(
            xs.transpose(0, 3, 2, 1).reshape(A, DC, 128, 2, N)
              .transpose(0, 2, 1, 3, 4)).astype(BF)
        dpt = f["dep_feature"][b].transpose(0, 2, 1).reshape(A, DC, 128, N)
        m["dp8"] = np.clip(np.ascontiguousarray(dpt.transpose(0, 2, 1, 3)),
                           -240, 240).astype(E4).reshape(
                               A, 128, DC // 2, 2, N)
        in_maps.append(m)
    return in_maps


def _install_ntff_shim():
    """Provide antenv.axon_hooks (absent in this image) so trace=True can
    drive NTFF capture through libaxon_pjrt.so."""
    if "antenv.axon_hooks" in sys.modules:
        return
    import antenv

    mod = types.ModuleType("antenv.axon_hooks")
    mod._hook = None
    mod.set_axon_ntff_profile_hook = lambda h: setattr(mod, "_hook", h)
    mod.get_axon_ntff_profile_hook = lambda: mod._hook
    sys.modules["antenv.axon_hooks"] = mod
    antenv.axon_hooks = mod

    so_path = "/opt/axon/libaxon_pjrt.so"
    try:
        lib = ctypes.CDLL(so_path)
    except OSError:
        return
    if not hasattr(lib, "axon_start_nrt_profile"):
        return
    lib.axon_start_nrt_profile.argtypes = [ctypes.POINTER(ctypes.c_int64),
                                           ctypes.c_size_t]
    lib.axon_start_nrt_profile.restype = ctypes.c_int64
    lib.axon_stop_nrt_profile.argtypes = [ctypes.c_char_p]
    lib.axon_stop_nrt_profile.restype = ctypes.c_int64

    @contextlib.contextmanager
    def _hook(output_dir, device_ids):
        import jax

        jax.devices()
        if device_ids:
            ids = (ctypes.c_int64 * len(device_ids))(*device_ids)
            rc = lib.axon_start_nrt_profile(ids, len(device_ids))
        else:
            rc = lib.axon_start_nrt_profile(None, 0)
        if rc != 0:
            raise RuntimeError(f"axon_start_nrt_profile rc={rc}")
        try:
            yield
        finally:
            n = lib.axon_stop_nrt_profile(str(output_dir).encode())
            print(f"profile: {n} file(s) written to {output_dir}")

    mod.set_axon_ntff_profile_hook(_hook)


def kernel(feature, dep_feature, aspect_feature, all_type_feature, fmask,
           Wq, bq, Wk, bk, Wv, bv, trans_W, trans_b, T1, W1_W, W1_b, comb_w,
           _profile=False, _tmpdir=None):
    global LAST_RESULTS
    inputs = dict(feature=feature, dep_feature=dep_feature,
                  aspect_feature=aspect_feature,
                  all_type_feature=all_type_feature, fmask=fmask, Wq=Wq,
                  bq=bq, Wk=Wk, bk=bk, Wv=Wv, bv=bv, trans_W=trans_W,
                  trans_b=trans_b, T1=T1, W1_W=W1_W, W1_b=W1_b,
                  comb_w=comb_w)
    nc = _build()
    in_maps = _prep_inputs(inputs)
    if _profile:
        _install_ntff_shim()
    res = run_bass_kernel_spmd(nc, in_maps, list(range(NCORES)),
                               trace=_profile, tmpdir=_tmpdir)
    LAST_RESULTS = res
    full = np.stack([res.results[c]["out"].T for c in range(NCORES)])
    return full.astype(np.float32)


# revision 11
# speedup vs baseline: 1.4263x; 1.2020x over previous
"""Bass/Tile TRN2 kernel for nn_Disen_GAT_For_Multi_Aspect (v3).

Contract: kernel(**inputs) takes FULL fp32 numpy inputs (keys as in
reference.setup_inputs()) and returns the FULL [B, A, H] fp32 output.

Strategy
--------
Data-parallel over batch B across the 8 cores (1 batch row / core, A=4
aspects per core).  The reference collapses algebraically:

  q = Wq^T asp + bq;  u = tA q; v = tB q; y = W1b v; a3 = W1a^T q
  w[a,k] = sum_{i,j} q[a,i] v[a,j] T1[i,j,k]
  G = Wk @ [q|w|y|u]                    (per aspect, [D,4])
  logit rows vs raw streams:  st = (Wk q).T_n, sx* = (Wk{q,w,y}).X_n,
                              sd = (Wk u).Dp_n
  V_W = Wv^T X + bv, V_T = Wv^T T + bv  ([H,N] per aspect)
  att_z[h] = sum_n att_n V_W[h,n] V_T[h,n]

PE-centric v3 layout:
 * T1 pass computes w DIRECTLY: 128 accumulating matmuls with rank-1
   lhsT chunks qv_j = q (.) v_j (fp8), rhs = T1[:, j-block] (fp8).
 * Per aspect the 5 logit rows land in ONE PSUM bank via tile_position
   col placement (X-rows@0, T-rows@32, Dp-rows@64, neg-row@96), padded
   to M=32 so every PSUM partition is written.  Row matmuls run in
   fp8 DoubleRow mode (K=256 chunk pairs): device-side single-copy
   casts of the bf16 X|T stream and of G to fp8.  One full-width bf16
   copy of the row bank, then softmax as: combo-matmul [128x3] -> ACT
   Exp(bias,scale,accum z) -> reciprocal -> alpha-broadcast matmul =
   att replicated on 128 partitions -> multiply+reduce vs V_W*V_T.
 * Streams: X,T bf16 (V path); rows read fp8 casts; Dp fp8; T1 fp8.
 * q-chain in bf16; constants split hot (q-chain) / cold (stream
   weights) so the first matmul issues as early as possible.
"""

import contextlib
import ctypes
import sys
import types

import numpy as np
import ml_dtypes

import concourse.bacc as bacc
import concourse.mybir as mybir
import concourse.tile as tile
from concourse.bass_utils import run_bass_kernel_spmd

B, A, N, D, H = 8, 4, 512, 1024, 128
SCALE = float(np.sqrt(H))
NCORES = 8
DC = D // H  # 8 contraction chunks of 128

F32 = mybir.dt.float32
BF16 = mybir.dt.bfloat16
F8 = mybir.dt.float8e4
BF = ml_dtypes.bfloat16
E4 = ml_dtypes.float8_e4m3fn
AF = mybir.ActivationFunctionType
OP = mybir.AluOpType
DR = mybir.MatmulPerfMode.DoubleRow

# cpackh (bf16, hot: q-chain weights) column layout
CH_WQ = 0              # [128, 8, 128] Wq chunk-packed
CH_ASP = 1024          # [128, 8, 4] aspect^T chunk-packed
CH_TAT = 1056          # trans_W[:H].T
CH_TBT = 1184          # trans_W[H:].T
CH_W1A = 1312          # W1_W[:H] (raw)
CH_W1BT = 1440         # W1_W[H:].T
CH_W = 1568
# cpackf (f32, small) column layout
CF_BQROW = 0           # rows 0-3: bq as a row [4, 128]
CF_BIAS = 128          # cols: bq|bk|bv|W1_b|trans_b
CF_COMBW = 133         # rows 0-2: comb_w column
CF_MASK4 = 134         # rows 0-3: eye(4) columns
CF_M01 = 138           # [1/S, 1/S, 0] column (rows 0-2)
CF_M2 = 139            # [0, 0, 1/S] column
CF_W = 140
# cpackb (bf16, cold: stream weights) column layout
CB_WKT = 0             # [128, 1024]  Wk^T
CB_WV = 1024           # [128, 8, 128] Wv chunk-packed
CB_MROW = 2048         # rows 0-3: fmask replicated [4, 512]
CB_COMBO = 2560        # [128, 3] combo matrix
CB_E0 = 2563           # [4, 32] one-hot row-0 picker (neg MM lhsT)
CB_ID4 = 2595          # [4, 4] identity
CB_W = 2600

LAST_RESULTS = None  # test harness peeks at this


def _build(ncores=NCORES):
    nc = bacc.Bacc("TRN2", target_bir_lowering=False, debug=False,
                   num_devices=ncores)

    xt = nc.dram_tensor("xt", [A, 128, DC, 2, N], BF16, kind="ExternalInput")
    dp8 = nc.dram_tensor("dp8", [A, 128, DC // 2, 2, N], F8,
                         kind="ExternalInput")
    t1f = nc.dram_tensor("t1f", [H, H * H], F8, kind="ExternalInput")
    cpackh = nc.dram_tensor("cpackh", [128, CH_W], BF16, kind="ExternalInput")
    cpackf = nc.dram_tensor("cpackf", [128, CF_W], F32, kind="ExternalInput")
    cpackb = nc.dram_tensor("cpackb", [128, CB_W], BF16, kind="ExternalInput")
    out = nc.dram_tensor("out", [H, A], F32, kind="ExternalOutput")

    inv_s = 1.0 / SCALE

    with tile.TileContext(nc) as tc:
        with (
            tc.tile_pool(name="const", bufs=1) as cp,
            tc.tile_pool(name="t1s", bufs=2) as tp,
            tc.tile_pool(name="xts", bufs=2) as xp,
            tc.tile_pool(name="work", bufs=2) as wp,
            tc.tile_pool(name="vzone", bufs=4, space="PSUM") as vps,
            tc.tile_pool(name="rzone", bufs=2, space="PSUM") as rps,
            tc.tile_pool(name="szone", bufs=2, space="PSUM") as sps,
        ):
            # ---- input DMAs, spread across the three DMA sequencers ---
            cph = cp.tile([128, CH_W], BF16, tag="cph")
            nc.sync.dma_start(out=cph, in_=cpackh.ap())
            cpf = cp.tile([128, CF_W], F32, tag="cpf")
            nc.scalar.dma_start(out=cpf, in_=cpackf.ap())
            t1sb = []
            for i in range(4):
                t = tp.tile([128, 4096], F8, tag="t1", bufs=4)
                eng = nc.sync if i % 2 == 0 else nc.scalar
                eng.dma_start(out=t, in_=t1f.ap()[:, 4096 * i:4096 * (i + 1)])
                t1sb.append(t)
            cpb = cp.tile([128, CB_W], BF16, tag="cpb")
            nc.scalar.dma_start(out=cpb, in_=cpackb.ap())
            xa_t = {}
            dp_t = {}
            xeng = [nc.gpsimd, nc.sync, nc.scalar, nc.gpsimd]
            deng = [nc.gpsimd, nc.gpsimd, nc.sync, nc.scalar]
            for a in range(A):
                xa = xp.tile([128, DC, 2, N], BF16, tag="xt", bufs=3)
                xeng[a].dma_start(out=xa, in_=xt.ap()[a])
                da = xp.tile([128, DC // 2, 2, N], F8, tag="dp")
                deng[a].dma_start(out=da, in_=dp8.ap()[a])
                xa_t[a] = xa
                dp_t[a] = da

            # ---- constant views ---------------------------------------
            wq_v = cph[:, CH_WQ:CH_WQ + DC * H].rearrange(
                "p (c h) -> p c h", c=DC)
            asp_v = cph[:, CH_ASP:CH_ASP + DC * A].rearrange(
                "p (c a) -> p c a", c=DC)
            tat_sb = cph[:, CH_TAT:CH_TAT + H]
            tbt_sb = cph[:, CH_TBT:CH_TBT + H]
            w1a_sb = cph[:, CH_W1A:CH_W1A + H]
            w1bt_sb = cph[:, CH_W1BT:CH_W1BT + H]
            bqrow = cpf[0:4, CF_BQROW:CF_BQROW + H]
            bq_c = cpf[:, CF_BIAS + 0:CF_BIAS + 1]
            bk_c = cpf[:, CF_BIAS + 1:CF_BIAS + 2]
            bv_c = cpf[:, CF_BIAS + 2:CF_BIAS + 3]
            b1_c = cpf[:, CF_BIAS + 3:CF_BIAS + 4]
            tb_c = cpf[:, CF_BIAS + 4:CF_BIAS + 5]
            combw3 = cpf[0:3, CF_COMBW:CF_COMBW + 1]
            mask4 = cpf[0:4, CF_MASK4:CF_MASK4 + 4]
            m01_c = cpf[0:3, CF_M01:CF_M01 + 1]
            m2_c = cpf[0:3, CF_M2:CF_M2 + 1]
            wkt_sb = cpb[:, CB_WKT:CB_WKT + D]
            wv_v = cpb[:, CB_WV:CB_WV + DC * H].rearrange(
                "p (c h) -> p c h", c=DC)
            mrow4 = cpb[0:4, CB_MROW:CB_MROW + N]
            combo_m = cpb[:, CB_COMBO:CB_COMBO + 3]
            id4 = cpb[0:4, CB_ID4:CB_ID4 + 4]

            ones_col = cp.tile([128, 1], F32, tag="ones_col")
            nc.vector.memset(ones_col, 1.0)
            ones3r = cp.tile([3, 128], BF16, tag="ones3r")
            nc.vector.memset(ones3r, 1.0)

            # neg rows: [32, N] with row 0 = -1e30*(1-m), rest zero
            negfull = cp.tile([32, N], BF16, tag="negfull")
            nc.vector.memset(negfull, 0.0)
            nc.vector.tensor_scalar(negfull[0:1, :], mrow4[0:1, :], 1e30,
                                    1e30, op0=OP.mult, op1=OP.subtract)

            # ---- q chain (bf16 matmuls, fp32 psum) --------------------
            ps_q = sps.tile([H, A], F32, tag="s")
            for c in range(DC):
                nc.tensor.matmul(ps_q, lhsT=wq_v[:, c, :], rhs=asp_v[:, c, :],
                                 start=(c == 0), stop=(c == DC - 1))
            q4 = cp.tile([H, A], F32, tag="q4")
            nc.scalar.activation(q4, ps_q, AF.Identity, bias=bq_c)
            q4b = cp.tile([H, A], BF16, tag="q4b")
            nc.vector.tensor_copy(q4b, q4)

            ps_qT = sps.tile([A, H], F32, tag="s")
            for c in range(DC):
                nc.tensor.matmul(ps_qT, lhsT=asp_v[:, c, :], rhs=wq_v[:, c, :],
                                 start=(c == 0), stop=(c == DC - 1))
            qTb = cp.tile([A, H], BF16, tag="qTb")
            nc.vector.tensor_tensor(qTb, ps_qT, bqrow, op=OP.add)

            ps_s = sps.tile([H, A], F32, tag="s")
            nc.tensor.matmul(ps_s, lhsT=tbt_sb, rhs=q4b, start=True, stop=True)
            v4 = cp.tile([H, A], F32, tag="v4")
            nc.vector.tensor_copy(v4, ps_s)
            v4b = cp.tile([H, A], BF16, tag="v4b")
            nc.vector.tensor_copy(v4b, ps_s)

            ps_vT = sps.tile([A, H], F32, tag="s")
            nc.tensor.matmul(ps_vT, lhsT=q4b, rhs=tbt_sb, start=True,
                             stop=True)
            vTb = cp.tile([A, H], BF16, tag="vTb")
            nc.vector.tensor_copy(vTb, ps_vT)

            # qwyu: aspect-major columns [q|w|y|u] per aspect, bf16
            qwyu = cp.tile([H, 16], BF16, tag="qwyu")
            qwv = qwyu.rearrange("p (a v) -> p a v", a=4)
            nc.vector.tensor_copy(qwv[:, :, 0], q4)

            ps_s = sps.tile([H, A], F32, tag="s")
            nc.tensor.matmul(ps_s, lhsT=tat_sb, rhs=q4b, start=True, stop=True)
            u4 = cp.tile([H, A], F32, tag="u4")
            nc.vector.tensor_copy(u4, ps_s)
            nc.vector.tensor_copy(qwv[:, :, 3], ps_s)

            ps_s = sps.tile([H, A], F32, tag="s")
            nc.tensor.matmul(ps_s, lhsT=w1bt_sb, rhs=v4b, start=True,
                             stop=True)
            y4 = cp.tile([H, A], F32, tag="y4")
            nc.vector.tensor_copy(y4, ps_s)
            nc.vector.tensor_copy(qwv[:, :, 2], ps_s)

            ps_s = sps.tile([H, A], F32, tag="s")
            nc.tensor.matmul(ps_s, lhsT=w1a_sb, rhs=q4b, start=True, stop=True)
            a3q = cp.tile([H, A], F32, tag="a3q")
            nc.vector.tensor_copy(a3q, ps_s)

            # ---- qv outer products (masked K=4), cast fp8 -------------
            ps_qv = sps.tile([128, 4 * H], F32, tag="s")
            for a in range(A):
                vTm = wp.tile([A, H], BF16, tag="vTm")
                nc.vector.tensor_scalar_mul(vTm, vTb, mask4[:, a:a + 1])
                nc.tensor.matmul(ps_qv[:, a * H:(a + 1) * H], lhsT=qTb,
                                 rhs=vTm, start=True, stop=True)
            qv8 = cp.tile([128, 4 * H], F8, tag="qv8")
            nc.vector.tensor_copy(qv8, ps_qv)
            qv8v = qv8.rearrange("p (a j) -> p j a", a=4)

            # ---- T1 pass: w[a,k] = sum_j qv_j . T1[:, j-block] --------
            ps_w = sps.tile([A, H], F32, tag="s")
            for j in range(H):
                nc.tensor.matmul(ps_w, lhsT=qv8v[:, j, :],
                                 rhs=t1sb[j // 32][:, (j % 32) * H:
                                                   (j % 32 + 1) * H],
                                 start=(j == 0), stop=(j == H - 1))
            wbf = cp.tile([A, H], BF16, tag="wbf")
            nc.vector.tensor_copy(wbf, ps_w)
            ps_tr = sps.tile([H, A], BF16, tag="s")
            nc.tensor.transpose(ps_tr, wbf, id4)
            nc.vector.tensor_copy(qwv[:, :, 1], ps_tr)

            # ---- scalar terms -> bias_all [3, A] ----------------------
            # groups: cbk | u.bk | w.bk | y.bk | a3.v | v.W1b | q.tb
            tmp28 = cp.tile([H, 28], F32, tag="tmp28")
            nc.vector.tensor_scalar_mul(tmp28[:, 0:4], q4, bk_c)
            nc.vector.tensor_scalar_mul(tmp28[:, 4:8], u4, bk_c)
            wcol = cp.tile([H, A], F32, tag="wcol")
            nc.vector.tensor_copy(wcol, ps_tr)
            nc.vector.tensor_scalar_mul(tmp28[:, 8:12], wcol, bk_c)
            nc.vector.tensor_scalar_mul(tmp28[:, 12:16], y4, bk_c)
            nc.vector.tensor_mul(tmp28[:, 16:20], a3q, v4)
            nc.vector.tensor_scalar_mul(tmp28[:, 20:24], v4, b1_c)
            nc.vector.tensor_scalar_mul(tmp28[:, 24:28], q4, tb_c)
            ps_c28 = sps.tile([1, 28], F32, tag="s")
            nc.tensor.matmul(ps_c28, lhsT=ones_col, rhs=tmp28,
                             start=True, stop=True)
            c28 = cp.tile([1, 28], F32, tag="c28")
            nc.vector.tensor_copy(c28, ps_c28)
            one13 = cp.tile([1, 3], F32, tag="one13")
            nc.vector.memset(one13, 1.0)
            ps_r3 = sps.tile([3, 28], F32, tag="s")
            nc.tensor.matmul(ps_r3, lhsT=one13, rhs=c28, start=True, stop=True)
            rep3 = cp.tile([3, 28], F32, tag="rep3")
            nc.vector.tensor_copy(rep3, ps_r3)
            cdw3 = cp.tile([3, A], F32, tag="cdw3")
            nc.vector.tensor_tensor(cdw3, rep3[:, 4:8], rep3[:, 8:12],
                                    op=OP.add)
            nc.vector.tensor_tensor(cdw3, cdw3, rep3[:, 12:16], op=OP.add)
            nc.vector.tensor_tensor(cdw3, cdw3, rep3[:, 16:20], op=OP.add)
            nc.vector.tensor_tensor(cdw3, cdw3, rep3[:, 20:24], op=OP.add)
            nc.vector.tensor_tensor(cdw3, cdw3, rep3[:, 24:28], op=OP.add)
            bias_all = cp.tile([3, A], F32, tag="bias_all")
            nc.vector.tensor_scalar_mul(bias_all, rep3[:, 0:4], m01_c)
            nc.vector.scalar_tensor_tensor(bias_all, cdw3, m2_c, bias_all,
                                           op0=OP.mult, op1=OP.add)

            # ---- G4 = Wk @ qwyu -> gall (zero-padded) + fp8 copy ------
            gall = cp.tile([128, DC, 48], BF16, tag="gall")
            nc.vector.memset(gall, 0.0)
            for c in range(DC):
                ps_g = sps.tile([128, 16], F32, tag="s")
                nc.tensor.matmul(ps_g, lhsT=wkt_sb[:, c * H:(c + 1) * H],
                                 rhs=qwyu, start=True, stop=True)
                nc.vector.tensor_copy(gall[:, c, 0:16], ps_g)
            g8 = cp.tile([128, DC, 48], F8, tag="g8")
            nc.vector.tensor_copy(g8, gall)
            g8p = g8.rearrange("p (c2 pair) f -> p c2 pair f", pair=2)

            # ---- per-aspect streams + finalization --------------------
            attz = cp.tile([H, A], F32, tag="attz")

            for a in range(A):
                xa, da = xa_t[a], dp_t[a]
                # row matmuls: X/T in bf16 (M=32-padded gall) into one
                # bank at col positions 0/32; Dp in fp8 DoubleRow in its
                # own bank.
                ps_rm = rps.tile([128, N], F32, tag="rows", bufs=1)
                for c in range(DC):
                    nc.tensor.matmul(ps_rm[0:32, :],
                                     lhsT=gall[:, c, 4 * a:4 * a + 32],
                                     rhs=xa[:, c, 0, :], start=(c == 0),
                                     stop=(c == DC - 1),
                                     tile_position=(0, 0))
                for c in range(DC):
                    nc.tensor.matmul(ps_rm[32:64, :],
                                     lhsT=gall[:, c, 4 * a:4 * a + 32],
                                     rhs=xa[:, c, 1, :], start=(c == 0),
                                     stop=(c == DC - 1),
                                     tile_position=(0, 32))
                ps_rd = rps.tile([32, N], F32, tag="rowsd", bufs=1)
                for c2 in range(DC // 2):
                    nc.tensor.matmul(ps_rd,
                                     lhsT=g8p[:, c2, :, 4 * a:4 * a + 32],
                                     rhs=da[:, c2, :, :], start=(c2 == 0),
                                     stop=(c2 == DC // 2 - 1),
                                     perf_mode=DR)

                # assemble rows bank: X@0, T@32, neg@64, Dp@96
                rows_bf = wp.tile([128, N], BF16, tag="rows_bf")
                nc.vector.tensor_copy(rows_bf[0:64, :], ps_rm[0:64, :])
                nc.scalar.copy(rows_bf[64:96, :], negfull)
                nc.vector.tensor_copy(rows_bf[96:128, :], ps_rd)
                ps_combo = sps.tile([3, N], F32, tag="s")
                nc.tensor.matmul(ps_combo, lhsT=combo_m, rhs=rows_bf,
                                 start=True, stop=True)
                e3 = wp.tile([3, N], BF16, tag="e3")
                z3 = wp.tile([3, 1], F32, tag="z3")
                nc.scalar.activation(e3, ps_combo, AF.Exp,
                                     bias=bias_all[:, a:a + 1], scale=inv_s,
                                     accum_out=z3)
                rz = wp.tile([3, 1], F32, tag="rz")
                nc.vector.reciprocal(rz, z3)
                alpha = wp.tile([3, 1], F32, tag="alpha")
                nc.vector.tensor_mul(alpha, rz, combw3)
                arep = wp.tile([3, H], BF16, tag="arep")
                nc.vector.tensor_scalar_mul(arep, ones3r, alpha)

                # V matmuls
                ps_vw = vps.tile([H, N], F32, tag="v")
                ps_vt = vps.tile([H, N], F32, tag="v")
                for c in range(DC):
                    nc.tensor.matmul(ps_vw, lhsT=wv_v[:, c, :],
                                     rhs=xa[:, c, 0, :], start=(c == 0),
                                     stop=(c == DC - 1))
                    nc.tensor.matmul(ps_vt, lhsT=wv_v[:, c, :],
                                     rhs=xa[:, c, 1, :], start=(c == 0),
                                     stop=(c == DC - 1))
                vv = wp.tile([H, 2 * N], F32, tag="vv")
                nc.scalar.activation(vv[:, 0:N], ps_vw, AF.Identity,
                                     bias=bv_c)
                nc.scalar.activation(vv[:, N:2 * N], ps_vt, AF.Identity,
                                     bias=bv_c)
                pprod = wp.tile([H, N], F32, tag="pprod")
                nc.vector.tensor_mul(pprod, vv[:, 0:N], vv[:, N:2 * N])

                ps_att = sps.tile([H, N], F32, tag="s")
                nc.tensor.matmul(ps_att, lhsT=arep, rhs=e3,
                                 start=True, stop=True)
                scr = wp.tile([H, N], F32, tag="scr")
                nc.vector.tensor_mul(scr, ps_att, pprod)
                nc.vector.tensor_reduce(attz[:, a:a + 1], scr,
                                        axis=mybir.AxisListType.X,
                                        op=OP.add)

            nc.sync.dma_start(out=out.ap(), in_=attz)

    nc.compile()
    return nc


def _prep_inputs(inputs):
    f = {k: np.asarray(v, dtype=np.float32) for k, v in inputs.items()}
    S = SCALE

    cpackh = np.zeros((128, CH_W), np.float32)
    cpackh[:, CH_WQ:CH_WQ + DC * H] = np.transpose(
        f["Wq"].reshape(DC, 128, H), (1, 0, 2)).reshape(128, DC * H)
    cpackh[:, CH_TAT:CH_TAT + H] = f["trans_W"][:H].T
    cpackh[:, CH_TBT:CH_TBT + H] = f["trans_W"][H:].T
    cpackh[:, CH_W1A:CH_W1A + H] = f["W1_W"][:H]
    cpackh[:, CH_W1BT:CH_W1BT + H] = f["W1_W"][H:].T

    cpackf = np.zeros((128, CF_W), np.float32)
    cpackf[0:4, CF_BQROW:CF_BQROW + H] = np.tile(f["bq"], (4, 1))
    for i, k in enumerate(("bq", "bk", "bv", "W1_b", "trans_b")):
        cpackf[:, CF_BIAS + i] = f[k]
    cpackf[0:3, CF_COMBW] = f["comb_w"]
    cpackf[0:4, CF_MASK4:CF_MASK4 + 4] = np.eye(4)
    cpackf[0:3, CF_M01] = [1.0 / S, 1.0 / S, 0.0]
    cpackf[0:3, CF_M2] = [0.0, 0.0, 1.0 / S]

    cpackb = np.zeros((128, CB_W), np.float32)
    cpackb[:, CB_WKT:CB_WKT + D] = f["Wk"].T
    cpackb[:, CB_WV:CB_WV + DC * H] = np.transpose(
        f["Wv"].reshape(DC, 128, H), (1, 0, 2)).reshape(128, DC * H)
    # combo matrix: ch0(TW): st@32, neg@64; ch1(Wi): sxq@0, neg@64;
    # ch2(DW): sxw@1, sxy@2, sd@99, neg@64
    cpackb[32, CB_COMBO + 0] = 1.0
    cpackb[64, CB_COMBO + 0] = 1.0
    cpackb[0, CB_COMBO + 1] = 1.0
    cpackb[64, CB_COMBO + 1] = 1.0
    cpackb[1, CB_COMBO + 2] = 1.0
    cpackb[2, CB_COMBO + 2] = 1.0
    cpackb[99, CB_COMBO + 2] = 1.0
    cpackb[64, CB_COMBO + 2] = 1.0
    cpackb[0, CB_E0] = 1.0
    cpackb[0:4, CB_ID4:CB_ID4 + 4] = np.eye(4)

    t1 = f["T1"].reshape(H, H * H)
    cpackh_bf = cpackh.astype(BF)
    t1_e4 = np.clip(t1, -240, 240).astype(E4)

    in_maps = []
    for b in range(NCORES):
        ch = cpackh_bf.copy()
        ch[:, CH_ASP:CH_ASP + DC * A] = np.transpose(
            f["aspect_feature"][b].T.reshape(DC, 128, A),
            (1, 0, 2)).reshape(128, DC * A).astype(BF)
        cb = cpackb.copy()
        cb[0:4, CB_MROW:CB_MROW + N] = np.tile(f["fmask"][b], (4, 1))
        m = {"t1f": t1_e4, "cpackh": ch, "cpackf": cpackf,
             "cpackb": cb.astype(BF)}
        xs = np.stack([f["feature"][b], f["all_type_feature"][b]], axis=2)
        # [A, N, 2, D] -> [A, 128(p), DC(c), 2, N]
        m["xt"] = np.ascontiguousarray(
            xs.transpose(0, 3, 2, 1).reshape(A, DC, 128, 2, N)
              .transpose(0, 2, 1, 3, 4)).astype(BF)
        dpt = f["dep_feature"][b].transpose(0, 2, 1).reshape(A, DC, 128, N)
        m["dp8"] = np.clip(np.ascontiguousarray(dpt.transpose(0, 2, 1, 3)),
                           -240, 240).astype(E4).reshape(
                               A, 128, DC // 2, 2, N)
        in_maps.append(m)
    return in_maps


def _install_ntff_shim():
    """Provide antenv.axon_hooks (absent in this image) so trace=True can
    drive NTFF capture through libaxon_pjrt.so."""
    if "antenv.axon_hooks" in sys.modules:
        return
    import antenv

    mod = types.ModuleType("antenv.axon_hooks")
    mod._hook = None
    mod.set_axon_ntff_profile_hook = lambda h: setattr(mod, "_hook", h)
    mod.get_axon_ntff_profile_hook = lambda: mod._hook
    sys.modules["antenv.axon_hooks"] = mod
    antenv.axon_hooks = mod

    so_path = "/opt/axon/libaxon_pjrt.so"
    try:
        lib = ctypes.CDLL(so_path)
    except OSError:
        return
    if not hasattr(lib, "axon_start_nrt_profile"):
        return
    lib.axon_start_nrt_profile.argtypes = [ctypes.POINTER(ctypes.c_int64),
                                           ctypes.c_size_t]
    lib.axon_start_nrt_profile.restype = ctypes.c_int64
    lib.axon_stop_nrt_profile.argtypes = [ctypes.c_char_p]
    lib.axon_stop_nrt_profile.restype = ctypes.c_int64

    @contextlib.contextmanager
    def _hook(output_dir, device_ids):
        import jax

        jax.devices()
        if device_ids:
            ids = (ctypes.c_int64 * len(device_ids))(*device_ids)
            rc = lib.axon_start_nrt_profile(ids, len(device_ids))
        else:
            rc = lib.axon_start_nrt_profile(None, 0)
        if rc != 0:
            raise RuntimeError(f"axon_start_nrt_profile rc={rc}")
        try:
            yield
        finally:
            n = lib.axon_stop_nrt_profile(str(output_dir).encode())
            print(f"profile: {n} file(s) written to {output_dir}")

    mod.set_axon_ntff_profile_hook(_hook)


def kernel(feature, dep_feature, aspect_feature, all_type_feature, fmask,
           Wq, bq, Wk, bk, Wv, bv, trans_W, trans_b, T1, W1_W, W1_b, comb_w,
           _profile=False, _tmpdir=None):
    global LAST_RESULTS
    inputs = dict(feature=feature, dep_feature=dep_feature,
                  aspect_feature=aspect_feature,
                  all_type_feature=all_type_feature, fmask=fmask, Wq=Wq,
                  bq=bq, Wk=Wk, bk=bk, Wv=Wv, bv=bv, trans_W=trans_W,
                  trans_b=trans_b, T1=T1, W1_W=W1_W, W1_b=W1_b,
                  comb_w=comb_w)
    nc = _build()
    in_maps = _prep_inputs(inputs)
    if _profile:
        _install_ntff_shim()
    res = run_bass_kernel_spmd(nc, in_maps, list(range(NCORES)),
                               trace=_profile, tmpdir=_tmpdir)
    LAST_RESULTS = res
    full = np.stack([res.results[c]["out"].T for c in range(NCORES)])
    return full.astype(np.float32)
